# revision 3
# baseline (speedup 1.0000x reference)
# Trainium2 Bass kernel for nn_DecoderBlock (masked self-attn + cross-attn +
# LFFN decoder block with "linear" softmax attention over the head dim).
#
# Sharding: data-parallel over batch — 16 batch elems / 8 cores = 2 per core.
# All weights replicated per core (bf16); activations stream per batch elem.
#
# Math per core/batch elem (validated against the jax reference in numpy):
#   per head: Q/K/V = x @ W[h]        ([s, dq] layout, s on partitions)
#   expQ/expK = exp((Q|K)/DQ**0.25)   (mask added to Q rows < 127 first)
#   V' = V * (1/rowsum(expK))         (folds K-softmax denominator)
#   A  = expK^T @ V'                  ([dq, dq])
#   softQ = expQ * (1/rowsum(expQ));  softQT = transpose(softQ)   [dq, s]
#   BmT = A^T @ softQT                ([dq, s])
#   out rows [128h:128h+128] = sum_j BmT[:, j::8].T @ Wo.T[128j:128j+128, :]
#     (replicates the module's raw [b,h,s,d] -> [b, s, h*d] view)
#   residual + layernorm in natural [s, D] layout; LFFN via transposed chain.
import numpy as np
import ml_dtypes

import concourse.bacc as bacc
import concourse.mybir as mybir
import concourse.tile as tile
from concourse.bass_utils import run_bass_kernel_spmd

H, D, DQ, BNK, HID = 8, 1024, 128, 512, 1024
B, S_T, S_M = 16, 1024, 2048
SCALE = DQ ** 0.25
EPS = 1e-5
NEG = -200.0
N_CORES = 8
BPC = B // N_CORES  # batch elems per core

f32 = mybir.dt.float32
bf16 = mybir.dt.bfloat16
f8 = mybir.dt.float8e4
DR = mybir.MatmulPerfMode.DoubleRow
AF = mybir.ActivationFunctionType
ALU = mybir.AluOpType
bf = ml_dtypes.bfloat16
f8np = ml_dtypes.float8_e4m3


def _build(affine: bool):
    nc = bacc.Bacc("TRN2", target_bir_lowering=False, debug=False,
                   enable_asserts=True, num_devices=N_CORES)

    dt_in = {}
    def din(name, shape, dt=bf16):
        dt_in[name] = nc.dram_tensor(name, list(shape), dt, kind="ExternalInput").ap()
        return dt_in[name]

    y0 = din("y0", [BPC, S_T, D], f32)
    y0T = din("y0T", [BPC, 8, 128, S_T])           # [b][kchunk][128, S_T] bf16
    memT = din("memT", [BPC, 8, 16, 128, 128])     # [b][kchunk][smtile][128,128]
    wqkv1 = din("wqkv1", [3, 2, 8, 128, 512])      # [qkv][hg][kchunk][128, 512]
    wqkv2 = din("wqkv2", [3, 2, 8, 128, 512])
    wo1t = din("wo1t", [8, 128, D])                # [j][128, D]
    wo2t = din("wo2t", [8, 128, D])
    e1t = din("e1t", [8, 4, 128, 128])             # [kchunk][bn_tile][128,128]
    d1t = din("d1t", [4, 8, 128, 128])             # [bn_chunk][hid_tile]
    e2t = din("e2t", [8, 4, 128, 128])             # [hid_chunk][bn_tile]
    d2t = din("d2t", [4, 128, D])                  # [bn_chunk][128, D]
    mask4 = din("mask4", [128, 512], f32)
    if affine:
        grep = din("grep", [6, 128, D], f32)       # g1,b1,g2,b2,g3,b3 replicated

    out = nc.dram_tensor("out", [BPC, S_T, D], f32, kind="ExternalOutput").ap()

    with tile.TileContext(nc) as tc:
        with tc.tile_pool(name="dram", bufs=1, space="DRAM") as dpool:
            y1d = dpool.tile([BPC, S_T, D], f32)
            y2d = dpool.tile([BPC, S_T, D], f32)

            with tc.tile_pool(name="consts", bufs=1) as cpool:
                maskt = cpool.tile([128, 512], f32, tag="maskt")
                nc.sync.dma_start(maskt[:], mask4[:])
                eps_t = cpool.tile([128, 1], f32, tag="eps_t")
                nc.vector.memset(eps_t[:], EPS)
                gb = None
                if affine:
                    gb = [cpool.tile([128, D], f32, tag=f"gb{i}", name=f"gb{i}") for i in range(6)]
                    for i in range(6):
                        nc.sync.dma_start(gb[i][:], grep[i])

                _phase_attn(nc, tc, b_iter=range(BPC), masked=True,
                            xq_nat=y0, xqT_dram=y0T, kvT_dram=None,
                            wqkv=wqkv1, wot=wo1t, n_kv=8, maskt=maskt,
                            y_next_d=y1d, gb=gb, gbi=0, eps_t=eps_t)
                _phase_attn(nc, tc, b_iter=range(BPC), masked=False,
                            xq_nat=y1d, xqT_dram=None, kvT_dram=memT,
                            wqkv=wqkv2, wot=wo2t, n_kv=16, maskt=None,
                            y_next_d=y2d, gb=gb, gbi=2, eps_t=eps_t)
                _phase_lffn(nc, tc, y2d, e1t, d1t, e2t, d2t, out, gb, 4, eps_t)

    nc.compile()
    return nc


def _layernorm_store(nc, pool, rsd, dst_dram, gb, gbi, eps_t=None, also_bf16=False):
    """LN over the free axis of rsd [128, D] f32 (g/b optional), write f32
    tile to dst_dram; optionally return a bf16 copy of the normed tile."""
    st6 = pool.tile([128, 2, 6], f32, tag="ln_st6")
    mv = pool.tile([128, 2], f32, tag="ln_mv")
    nc.vector.bn_stats(st6[:, 0, :], rsd[:, 0:512])
    nc.vector.bn_stats(st6[:, 1, :], rsd[:, 512:1024])
    nc.vector.bn_aggr(mv[:], st6[:])
    sd = pool.tile([128, 1], f32, tag="ln_sd")
    nc.scalar.activation(sd[:], mv[:, 1:2], AF.Sqrt, bias=eps_t[:])
    rstd = pool.tile([128, 1], f32, tag="ln_rstd")
    nc.vector.reciprocal(rstd[:], sd[:])
    cneg = pool.tile([128, 1], f32, tag="ln_cneg")
    nc.vector.scalar_tensor_tensor(
        out=cneg[:], in0=mv[:, 0:1], scalar=-1.0, in1=rstd[:],
        op0=ALU.mult, op1=ALU.mult)
    yt = pool.tile([128, D], f32, tag="ln_out")
    nc.scalar.activation(yt[:], rsd[:], AF.Identity, scale=rstd[:], bias=cneg[:])
    if gb is not None:
        g_t, b_t = gb[gbi], gb[gbi + 1]
        nc.vector.tensor_tensor(out=yt[:], in0=yt[:], in1=g_t[:], op=ALU.mult)
        nc.vector.tensor_tensor(out=yt[:], in0=yt[:], in1=b_t[:], op=ALU.add)
    nc.sync.dma_start(dst_dram, yt[:])
    if also_bf16:
        yb = pool.tile([128, D], bf16, tag="ln_out_bf")
        nc.vector.tensor_copy(yb[:], yt[:])
        return yb
    return None


def _phase_attn(nc, tc, b_iter, masked, xq_nat, xqT_dram, kvT_dram,
                wqkv, wot, n_kv, maskt, y_next_d, gb, gbi, eps_t=None):
    """One attention phase (self or cross) for all batch elems."""
    with tc.tile_pool(name="attn_sb", bufs=1) as sb:
        # weights resident: wqkv rhs tiles [hg][k] for q/k/v + wot chunks
        wq_s, wk_s, wv_s = ([[None] * 8 for _ in range(2)] for _ in range(3))
        for hg in range(2):
            for k in range(8):
                for pi, ws in ((0, wq_s), (1, wk_s), (2, wv_s)):
                    t = sb.tile([128, 512], bf16, tag=f"w{pi}_{hg}_{k}")
                    nc.sync.dma_start(t[:], wqkv[pi, hg, k])
                    ws[hg][k] = t
        wot_s = []
        for j in range(8):
            t = sb.tile([128, D], bf16, tag=f"wot{j}")
            wot_s.append(t)
        wot_loaded = [False]

        for b in b_iter:
            # xqT tiles (lhsT for Q proj, and for self-attn also K/V)
            xqT = []
            if xqT_dram is not None:
                for k in range(8):
                    t = sb.tile([128, S_T], bf16, tag=f"xqT{k}")
                    nc.sync.dma_start(t[:], xqT_dram[b, k])
                    xqT.append(t)
            else:
                # rebuild transposed bf16 x from the natural f32 dram tensor
                for k in range(8):
                    xqT.append(sb.tile([128, S_T], bf16, tag=f"xqT{k}", name=f"xqT{k}"))
                for st in range(8):
                    nat = sb.tile([128, D], f32, tag="xq_nat_ld", bufs=2)
                    nc.sync.dma_start(nat[:], xq_nat[b, 128 * st:128 * (st + 1), :])
                    natb = sb.tile([128, D], bf16, tag="xq_nat_bf", bufs=2)
                    nc.vector.tensor_copy(natb[:], nat[:])
                    for k in range(8):
                        nc.sync.dma_start_transpose(
                            xqT[k][:, 128 * st:128 * (st + 1)],
                            natb[:, 128 * k:128 * (k + 1)])

            for hg in range(2):
                # ---- stage A: K/V projections + evac + A accumulation ----
                expk = sb.tile([128, n_kv, 512], bf16, tag="expk")
                expv = sb.tile([128, n_kv, 512], bf16, tag="expv")
                with tc.tile_pool(name="ps_a", bufs=1, space="PSUM") as psa:
                    for sm in range(n_kv):
                        kps = psa.tile([128, 512], f32, tag="kv", bufs=6)
                        vps = psa.tile([128, 512], f32, tag="kv", bufs=6)
                        for k in range(8):
                            if kvT_dram is None:
                                lhsT = xqT[k][:, 128 * sm:128 * (sm + 1)]
                            else:
                                lt = sb.tile([128, 128], bf16, tag="memlhs", bufs=4)
                                nc.sync.dma_start(lt[:], kvT_dram[b, k, sm])
                                lhsT = lt[:]
                            nc.tensor.matmul(kps[:], lhsT, wk_s[hg][k][:],
                                             start=(k == 0), stop=(k == 7))
                            nc.tensor.matmul(vps[:], lhsT, wv_s[hg][k][:],
                                             start=(k == 0), stop=(k == 7))
                        # evac: expK (bf16) + per-head rowsums; V' = V/rowsumK
                        nc.scalar.activation(
                            expk[:, sm, :], kps[:], AF.Exp, scale=1.0 / SCALE)
                        krs = sb.tile([128, 4], f32, tag="krs", bufs=2)
                        nc.vector.tensor_reduce(
                            out=krs[:],
                            in_=expk[:, sm, :].rearrange("p (h q) -> p h q", h=4),
                            axis=mybir.AxisListType.X, op=ALU.add)
                        krr = sb.tile([128, 4], f32, tag="krr", bufs=2)
                        nc.vector.reciprocal(krr[:], krs[:])
                        nc.vector.tensor_tensor(
                            out=expv[:, sm, :].rearrange("p (h q) -> p h q", h=4),
                            in0=vps[:].rearrange("p (h q) -> p h q", h=4),
                            in1=krr[:].unsqueeze(2).broadcast_to([128, 4, 128]),
                            op=ALU.mult)
                    # A for the 4 heads of this hg, packed in one psum bank
                    aps = psa.tile([128, 512], f32, tag="aps", bufs=2)
                    for hi in range(4):
                        for sm in range(n_kv):
                            nc.tensor.matmul(
                                aps[:, 128 * hi:128 * (hi + 1)],
                                expk[:, sm, 128 * hi:128 * (hi + 1)],
                                expv[:, sm, 128 * hi:128 * (hi + 1)],
                                start=(sm == 0), stop=(sm == n_kv - 1))
                    asb = sb.tile([128, 512], bf16, tag="asb")
                    nc.vector.tensor_copy(asb[:], aps[:])

                # ---- stage B: Q proj + softmax + transpose ----
                softqT = sb.tile([128, 4, S_T], bf16, tag="softqT")
                with tc.tile_pool(name="ps_b", bufs=1, space="PSUM") as psb:
                    for st in range(8):
                        qps = psb.tile([128, 512], f32, tag="qps", bufs=2)
                        for k in range(8):
                            nc.tensor.matmul(
                                qps[:], xqT[k][:, 128 * st:128 * (st + 1)],
                                wq_s[hg][k][:], start=(k == 0), stop=(k == 7))
                        if masked and st == 0:
                            nc.vector.tensor_tensor(
                                out=qps[:], in0=qps[:], in1=maskt[:], op=ALU.add)
                        eq = sb.tile([128, 512], f32, tag="eq", bufs=2)
                        nc.scalar.activation(eq[:], qps[:], AF.Exp, scale=1.0 / SCALE)
                        qrs = sb.tile([128, 4], f32, tag="qrs", bufs=2)
                        nc.vector.tensor_reduce(
                            out=qrs[:], in_=eq[:].rearrange("p (h q) -> p h q", h=4),
                            axis=mybir.AxisListType.X, op=ALU.add)
                        qrr = sb.tile([128, 4], f32, tag="qrr", bufs=2)
                        nc.vector.reciprocal(qrr[:], qrs[:])
                        sq = sb.tile([128, 4, 128], bf16, tag="sq", bufs=2)
                        nc.vector.tensor_tensor(
                            out=sq[:], in0=eq[:].rearrange("p (h q) -> p h q", h=4),
                            in1=qrr[:].unsqueeze(2).broadcast_to([128, 4, 128]),
                            op=ALU.mult)
                        for hi in range(4):
                            eng = nc.scalar if hi % 2 else nc.sync
                            eng.dma_start_transpose(
                                softqT[:, hi, 128 * st:128 * (st + 1)],
                                sq[:, hi, :])

                    # ---- stage C: Bm, Wo, residual + LN per head ----
                    if not wot_loaded[0]:
                        wot_loaded[0] = True
                        for j in range(8):
                            nc.sync.dma_start(wot_s[j][:], wot[j])
                    for hi in range(4):
                        hb = 4 * hg + hi  # head == output s-tile block
                        bmt = psb.tile([128, S_T], f32, tag="bmt")
                        nc.tensor.matmul(bmt[:, 0:512], asb[:, 128 * hi:128 * (hi + 1)],
                                         softqT[:, hi, 0:512])
                        nc.tensor.matmul(bmt[:, 512:1024], asb[:, 128 * hi:128 * (hi + 1)],
                                         softqT[:, hi, 512:1024])
                        bms = sb.tile([128, S_T], bf16, tag="bms", bufs=2)
                        nc.vector.tensor_copy(bms[:], bmt[:])
                        ops = psb.tile([128, D], f32, tag="ops", bufs=2)
                        for j in range(8):
                            for nh in range(2):
                                nc.tensor.matmul(
                                    ops[:, 512 * nh:512 * (nh + 1)],
                                    bms[:, j::8],
                                    wot_s[j][:, 512 * nh:512 * (nh + 1)],
                                    start=(j == 0), stop=(j == 7))
                        nat = sb.tile([128, D], f32, tag="res_nat", bufs=2)
                        nc.sync.dma_start(nat[:], xq_nat[b, 128 * hb:128 * (hb + 1), :])
                        rsd = sb.tile([128, D], f32, tag="rsd", bufs=2)
                        nc.vector.tensor_tensor(out=rsd[:], in0=ops[:], in1=nat[:],
                                                op=ALU.add)
                        _layernorm_store(
                            nc, sb, rsd, y_next_d[b, 128 * hb:128 * (hb + 1), :],
                            gb, gbi, eps_t)


def _phase_lffn(nc, tc, y2d, e1t, d1t, e2t, d2t, out, gb, gbi, eps_t=None):
    with tc.tile_pool(name="ffn_sb", bufs=1) as sb:
        e1s = [[None] * 4 for _ in range(8)]
        d1s = [[None] * 8 for _ in range(4)]
        e2s = [[None] * 4 for _ in range(8)]
        d2s = []
        for k in range(8):
            for t_ in range(4):
                e1s[k][t_] = sb.tile([128, 128], bf16, tag=f"e1_{k}_{t_}", name=f"e1_{k}_{t_}")
                nc.sync.dma_start(e1s[k][t_][:], e1t[k, t_])
                e2s[k][t_] = sb.tile([128, 128], bf16, tag=f"e2_{k}_{t_}", name=f"e2_{k}_{t_}")
                nc.sync.dma_start(e2s[k][t_][:], e2t[k, t_])
        for k in range(4):
            for t_ in range(8):
                d1s[k][t_] = sb.tile([128, 128], bf16, tag=f"d1_{k}_{t_}", name=f"d1_{k}_{t_}")
                nc.sync.dma_start(d1s[k][t_][:], d1t[k, t_])
            t = sb.tile([128, D], bf16, tag=f"d2_{k}")
            nc.sync.dma_start(t[:], d2t[k])
            d2s.append(t)

        for b in range(BPC):
            # y2T bf16 tiles rebuilt from y2 dram
            y2T = [sb.tile([128, S_T], bf16, tag=f"y2T{k}", name=f"y2T{k}") for k in range(8)]
            for st in range(8):
                nat = sb.tile([128, D], f32, tag="y2_nat_ld", bufs=2)
                nc.sync.dma_start(nat[:], y2d[b, 128 * st:128 * (st + 1), :])
                natb = sb.tile([128, D], bf16, tag="y2_nat_bf", bufs=2)
                nc.vector.tensor_copy(natb[:], nat[:])
                for k in range(8):
                    nc.sync.dma_start_transpose(
                        y2T[k][:, 128 * st:128 * (st + 1)],
                        natb[:, 128 * k:128 * (k + 1)])

            # h1T = E1 @ y2T  [BN(4 tiles), S_T]
            h1T = [sb.tile([128, S_T], bf16, tag=f"h1T{t_}", name=f"h1T{t_}") for t_ in range(4)]
            with tc.tile_pool(name="ps_f1", bufs=1, space="PSUM") as ps:
                for t_ in range(4):
                    acc = ps.tile([128, S_T], f32, tag="acc", bufs=3)
                    for nh in range(2):
                        for k in range(8):
                            nc.tensor.matmul(
                                acc[:, 512 * nh:512 * (nh + 1)], e1s[k][t_][:],
                                y2T[k][:, 512 * nh:512 * (nh + 1)],
                                start=(k == 0), stop=(k == 7))
                    nc.vector.tensor_copy(h1T[t_][:], acc[:])
            # h2T = D1 @ h1T -> silu -> swT  [HID(8 tiles), S_T]
            swT = [sb.tile([128, S_T], bf16, tag=f"swT{t_}", name=f"swT{t_}") for t_ in range(8)]
            with tc.tile_pool(name="ps_f2", bufs=1, space="PSUM") as ps:
                for t_ in range(8):
                    acc = ps.tile([128, S_T], f32, tag="acc", bufs=3)
                    for nh in range(2):
                        for k in range(4):
                            nc.tensor.matmul(
                                acc[:, 512 * nh:512 * (nh + 1)], d1s[k][t_][:],
                                h1T[k][:, 512 * nh:512 * (nh + 1)],
                                start=(k == 0), stop=(k == 3))
                    nc.scalar.activation(swT[t_][:], acc[:], AF.Silu)
            # g1T = E2 @ swT  [BN(4 tiles), S_T]
            g1T = [sb.tile([128, S_T], bf16, tag=f"g1T{t_}", name=f"g1T{t_}") for t_ in range(4)]
            with tc.tile_pool(name="ps_f3", bufs=1, space="PSUM") as ps:
                for t_ in range(4):
                    acc = ps.tile([128, S_T], f32, tag="acc", bufs=3)
                    for nh in range(2):
                        for k in range(8):
                            nc.tensor.matmul(
                                acc[:, 512 * nh:512 * (nh + 1)], e2s[k][t_][:],
                                swT[k][:, 512 * nh:512 * (nh + 1)],
                                start=(k == 0), stop=(k == 7))
                    nc.vector.tensor_copy(g1T[t_][:], acc[:])
            # ffn[st] = g1T[:, st].T @ D2T ; residual with y2, LN3 -> out
            with tc.tile_pool(name="ps_f4", bufs=1, space="PSUM") as ps:
                for st in range(8):
                    acc = ps.tile([128, D], f32, tag="acc", bufs=3)
                    for nh in range(2):
                        for k in range(4):
                            nc.tensor.matmul(
                                acc[:, 512 * nh:512 * (nh + 1)],
                                g1T[k][:, 128 * st:128 * (st + 1)],
                                d2s[k][:, 512 * nh:512 * (nh + 1)],
                                start=(k == 0), stop=(k == 3))
                    nat = sb.tile([128, D], f32, tag="y2res", bufs=2)
                    nc.sync.dma_start(nat[:], y2d[b, 128 * st:128 * (st + 1), :])
                    rsd = sb.tile([128, D], f32, tag="rsd", bufs=2)
                    nc.vector.tensor_tensor(out=rsd[:], in0=acc[:], in1=nat[:],
                                            op=ALU.add)
                    _layernorm_store(nc, sb, rsd,
                                     out[b, 128 * st:128 * (st + 1), :], gb, gbi,
                                     eps_t)


_CACHE = {}


def _prep_host(inputs):
    """Convert/transpose/tile weights + activations per the kernel layout."""
    g = {k: np.asarray(v) for k, v in inputs.items()}
    affine = not (
        np.all(g["g1"] == 1) and np.all(g["g2"] == 1) and np.all(g["g3"] == 1)
        and np.all(g["b1"] == 0) and np.all(g["b2"] == 0) and np.all(g["b3"] == 0))

    def wqkv_pack(q, k, v):
        # [H, D, DQ] -> [3][hg=2][kchunk=8][128, 512] (4 heads concat)
        def onev2(w):
            arr = np.empty((2, 8, 128, 512), np.float32)
            for hg in range(2):
                for kc in range(8):
                    cols = [w[4 * hg + hi, 128 * kc:128 * (kc + 1), :] for hi in range(4)]
                    arr[hg, kc] = np.concatenate(cols, axis=1)
            return arr
        return np.stack([onev2(q), onev2(k), onev2(v)]).astype(bf)

    host = {}
    host["wqkv1"] = wqkv_pack(g["Wq1"], g["Wk1"], g["Wv1"])
    host["wqkv2"] = wqkv_pack(g["Wq2"], g["Wk2"], g["Wv2"])
    host["wo1t"] = np.ascontiguousarray(g["Wo1"].T).reshape(8, 128, D).astype(bf)
    host["wo2t"] = np.ascontiguousarray(g["Wo2"].T).reshape(8, 128, D).astype(bf)
    host["e1t"] = np.ascontiguousarray(
        g["E1"].T).reshape(8, 128, 4, 128).transpose(0, 2, 1, 3).astype(bf)
    host["d1t"] = np.ascontiguousarray(
        g["D1"].T).reshape(4, 128, 8, 128).transpose(0, 2, 1, 3).astype(bf)
    host["e2t"] = np.ascontiguousarray(
        g["E2"].T).reshape(8, 128, 4, 128).transpose(0, 2, 1, 3).astype(bf)
    host["d2t"] = np.ascontiguousarray(g["D2"].T).reshape(4, 128, D).astype(bf)
    mask = np.where(np.arange(DQ)[None, :] <= np.arange(128)[:, None],
                    0.0, NEG).astype(np.float32)
    host["mask4"] = np.tile(mask, (1, 4))
    if affine:
        host["grep"] = np.stack([
            np.broadcast_to(g[n].astype(np.float32), (128, D))
            for n in ("g1", "b1", "g2", "b2", "g3", "b3")]).copy()

    in_maps = []
    y = g["y"].astype(np.float32)
    mem = g["mem"].astype(np.float32)
    for c in range(N_CORES):
        sl = slice(BPC * c, BPC * (c + 1))
        m = dict(host)
        m["y0"] = np.ascontiguousarray(y[sl])
        yT = np.ascontiguousarray(y[sl].transpose(0, 2, 1)).astype(bf)
        m["y0T"] = np.ascontiguousarray(yT.reshape(BPC, 8, 128, S_T))
        mT = np.ascontiguousarray(mem[sl].transpose(0, 2, 1)).astype(bf)
        m["memT"] = np.ascontiguousarray(
            mT.reshape(BPC, 8, 128, 16, 128).transpose(0, 1, 3, 2, 4))
        in_maps.append(m)
    return in_maps, affine


def kernel(**inputs):
    in_maps, affine = _prep_host(inputs)
    if affine not in _CACHE:
        _CACHE[affine] = _build(affine)
    nc = _CACHE[affine]
    res = run_bass_kernel_spmd(nc, in_maps, list(range(N_CORES)))
    return np.concatenate([r["out"] for r in res.results], axis=0)


if __name__ == "__main__":
    rng = np.random.default_rng(0)
    ins = {
        "mem": rng.standard_normal((B, S_M, D), dtype=np.float32),
        "y": rng.standard_normal((B, S_T, D), dtype=np.float32),
        **{k: (rng.standard_normal(s, dtype=np.float32) * 0.02).astype(np.float32)
           for k, s in {
               "Wq1": (H, D, DQ), "Wk1": (H, D, DQ), "Wv1": (H, D, DQ),
               "Wo1": (D, D), "Wq2": (H, D, DQ), "Wk2": (H, D, DQ),
               "Wv2": (H, D, DQ), "Wo2": (D, D), "E1": (BNK, D),
               "D1": (HID, BNK), "E2": (BNK, HID), "D2": (D, BNK)}.items()},
        "g1": np.ones(D, np.float32), "b1": np.zeros(D, np.float32),
        "g2": np.ones(D, np.float32), "b2": np.zeros(D, np.float32),
        "g3": np.ones(D, np.float32), "b3": np.zeros(D, np.float32),
    }
    o = kernel(**ins)
    print("out", o.shape, o.dtype, np.abs(o).mean())

